# revision 1
# baseline (speedup 1.0000x reference)
"""Trainium2 Bass kernel for BaseRelationNetwork forward pass.

Reference computation (per batch row b):
    pairs (i<j) of C=16 channels, P=120 pairs
    h1 = relu(concat(x_i, x_j) @ W1 + b1)      # W1 [2F, H]
    h2 = relu(h1 @ W2 + b2)
    out = mean_p(h2 @ W3 + b3)                 # [B, H]

Algebraic restructuring used here:
  1. W1 splits into W1a (top F rows, applied to x_i) and W1b (bottom F rows,
     applied to x_j). ya = x @ W1a and yb = x @ W1b are computed once per
     channel (C matmuls) instead of per pair (P matmuls): 7.5x less PE work.
     h1[p=(i,j)] = relu(ya[i] + yb[j] + b1) is a cheap DVE gather-add.
  2. mean over pairs commutes with the affine layer 3:
     out = (mean_p h2) @ W3 + b3. Layer 3 runs on the pair-mean only.

Sharding: data-parallel over batch. 512 rows / 8 cores = 64 rows per core.
Weights replicated. Host pre-transposes x to feature-major layout with
token = half*512 + c*32 + b (batch split in two halves of 32) so the
pipeline (layer-1 matmul -> pair-add -> layer-2 -> accumulate) runs as two
overlapping chunks; the 1/P mean scale is folded into W3 and the biases
are packed into one [128, 6] tile on the host.

Matmuls run in float32r (fast fp32 mode, reduced mantissa): full PE rate
when the moving free dim >= 256, ~1e-4 output error vs exact fp32.

DMA strategy: big loads (x, W1) go through gpsimd (SWDGE) as a few large
multi-tile transfers - the HWDGE queue serializes ~0.6us per dma_start, so
many small sync-engine DMAs throttle the front of the kernel.
"""

import contextlib
import sys

if "/opt/trn_rl_repo" not in sys.path:
    sys.path.insert(0, "/opt/trn_rl_repo")

import numpy as np

import concourse.bass as bass
import concourse.mybir as mybir
import concourse.tile as tile
from concourse import bacc
from concourse.bass_utils import run_bass_kernel_spmd

# Problem shape (hardcoded per contract).
B, C, F, H = 512, 16, 1024, 256
N_CORES = 8
BL = B // N_CORES          # 64 local batch rows per core
P = C * (C - 1) // 2       # 120 pairs
NH = 4                     # batch chunks per core (chunked pipeline)
BH = BL // NH              # 32 rows per half
TOK = BL * C               # 1024 tokens per core
HTOK = BH * C              # 512 tokens per half, token = half*512 + c*32 + b
F32 = mybir.dt.float32
F32R = mybir.dt.float32r

KT1 = F // 128             # 8 k-tiles for layer-1 contraction
KQ = 4                     # k-tiles per merged x DMA
PPG = 30                   # pairs per stage-C sub-group
GW = PPG * BH              # stage-C sub-group width: 480 columns
NG = P // PPG              # 8 stage-C sub-groups per half
NSP = NG // 2              # 4 double-width (960-col) stage-C groups per half

# pair enumeration: for i in 0..C-2, j in i+1..C-1, p consecutive
PAIR_BASE = [0] * C
for _i in range(1, C):
    PAIR_BASE[_i] = PAIR_BASE[_i - 1] + (C - 1 - (_i - 1))

AF = mybir.ActivationFunctionType
ALU = mybir.AluOpType


def build_module(loop_iters: int = 1):
    nc = bacc.Bacc("TRN2", target_bir_lowering=False, debug=True)

    xt_d = nc.dram_tensor("xt", [F, TOK], F32R, kind="ExternalInput")
    w1_d = nc.dram_tensor("w1", [2 * F, H], F32R, kind="ExternalInput")
    w2_d = nc.dram_tensor("w2", [H, H], F32R, kind="ExternalInput")
    w3_d = nc.dram_tensor("w3", [H, H], F32, kind="ExternalInput")
    bp_d = nc.dram_tensor("bias_pack", [128, 6], F32, kind="ExternalInput")
    id_d = nc.dram_tensor("ident", [128, 128], F32R, kind="ExternalInput")
    out_d = nc.dram_tensor("outT", [H, BL], F32, kind="ExternalOutput")

    with tile.TileContext(nc) as tc:
        with (
            tc.tile_pool(name="xpool", bufs=1) as xpool,
            tc.tile_pool(name="wpool", bufs=1) as wpool,
            tc.tile_pool(name="ypool", bufs=1) as ypool,
            tc.tile_pool(name="hpool", bufs=1) as hpool,
            tc.tile_pool(name="spool", bufs=1) as spool,
            tc.tile_pool(name="psA", bufs=4, space="PSUM") as psA_pool,
            tc.tile_pool(name="psC", bufs=2, space="PSUM") as psC_pool,
        ):
            loop_cm = (
                tc.For_i(0, loop_iters, 1)
                if loop_iters > 1
                else contextlib.nullcontext()
            )
            with loop_cm:
                # big tiles
                xts = xpool.tile([128, KT1, TOK], F32R, tag="xts", name="xts")
                w1big = wpool.tile([128, 2 * KT1, H], F32R, tag="w1big", name="w1big")
                w2t = wpool.tile([128, 2, H], F32R, tag="w2t", name="w2t")
                w3t = wpool.tile([128, 2, H], F32, tag="w3t", name="w3t")
                bp = wpool.tile([128, 6], F32, tag="bp", name="bp")
                idt = wpool.tile([128, 128], F32R, tag="idt", name="idt")
                # y_all free layout: [m(4), chunk(NH), c(C), b(BH)]
                y_all = ypool.tile([128, 4, TOK], F32, tag="y_all", name="y_all")
                # h1 free layout: [t(2), half(NH), p(P), b(BH)]
                h1all = hpool.tile(
                    [128, 2, NH * P * BH], F32R, tag="h1all", name="h1all"
                )
                h2sb = [
                    [
                        spool.tile(
                            [128, GW * NG], F32R,
                            tag=f"h2_{m}_{par}", name=f"h2_{m}_{par}",
                        )
                        for par in range(2)
                    ]
                    for m in range(2)
                ]
                m2 = [
                    spool.tile([128, BL], F32, tag=f"m2_{m}", name=f"m2_{m}")
                    for m in range(2)
                ]
                osb = spool.tile([128, 2, BL], F32, tag="osb", name="osb")

                def bias(nm, t):
                    idx = {"b1": 0, "b2": 2, "b3": 4}[nm] + t
                    return bp[:, idx : idx + 1]

                # W1 rows viewed [16 ktiles, 128, H] -> SBUF [128, k, H]
                w1v = w1_d.rearrange("(k p) h -> p k h", p=128)
                xtv = xt_d.rearrange("(k p) t -> p k t", p=128)

                def hs(half):
                    return slice(half * HTOK, (half + 1) * HTOK)

                # ---- DMA order: bias first (ya copies need b1), W1 quads +
                # x chunk 0, then w2 (stage C), remaining x chunks, w3 last ----
                nc.sync.dma_start(out=bp[:], in_=bp_d[:])
                nc.sync.dma_start(out=idt[:], in_=id_d[:])
                for q in range(2):
                    ks = slice(q * KQ, (q + 1) * KQ)
                    kbs = slice(KT1 + q * KQ, KT1 + (q + 1) * KQ)
                    nc.gpsimd.dma_start(out=w1big[:, ks, :], in_=w1v[:, ks, :])
                    nc.gpsimd.dma_start(out=w1big[:, kbs, :], in_=w1v[:, kbs, :])
                    nc.gpsimd.dma_start(
                        out=xts[:, ks, hs(0)], in_=xtv[:, ks, hs(0)]
                    )
                nc.sync.dma_start(
                    out=w2t[:], in_=w2_d.rearrange("(k p) h -> p k h", p=128)
                )
                for ch in range(1, NH):
                    nc.gpsimd.dma_start(
                        out=xts[:, :, hs(ch)], in_=xtv[:, :, hs(ch)]
                    )
                nc.sync.dma_start(
                    out=w3t[:], in_=w3_d.rearrange("(k p) h -> p k h", p=128)
                )

                def flush_acc(p):
                    ph, ppar = p
                    for m in range(2):
                        # sum the 4 su-blocks on PE: identity pass-through
                        # matmuls accumulating in PSUM (PE has slack)
                        psr = psC_pool.tile(
                            [128, GW], F32, tag="psC", name=f"psR_{ph}_{m}"
                        )
                        for su in range(2 * NSP):
                            nc.tensor.matmul(
                                psr[:],
                                idt[:],
                                h2sb[m][ppar][:, su * GW : (su + 1) * GW],
                                start=(su == 0),
                                stop=(su == 2 * NSP - 1),
                            )
                        # then reduce over p only: [128, b, p] view, 480 reads
                        v = psr.rearrange("q (pp b) -> q pp b", b=BH).transpose(
                            [0, 2, 1]
                        )
                        nc.vector.tensor_reduce(
                            m2[m][:, ph * BH : (ph + 1) * BH],
                            v,
                            mybir.AxisListType.X,
                            ALU.add,
                        )

                # PE warm-up while DMAs stream: ~10 dummy matmuls on the bias
                # tile into a psC-pool slot (free until stage C starts ~18us)
                warm = psC_pool.tile([128, 1024], F32, tag="psC", name="warm")
                for _ in range(10):
                    nc.tensor.matmul(
                        warm[:1, :256],
                        bp[:, 0:1],
                        bp[:, 0:1].broadcast_to([128, 256]),
                        start=True,
                        stop=True,
                    )

                pend = None
                for half in range(NH):
                    # ---- stage A (k-outer): matmuls for this half ----
                    psA = {
                        m: psA_pool.tile(
                            [128, HTOK], F32, tag="psA", name=f"psA_{half}_{m}"
                        )
                        for m in range(4)
                    }
                    for k in range(KT1):
                        for m in (0, 2, 1, 3):
                            w_half, ht = divmod(m, 2)
                            nc.tensor.matmul(
                                psA[m][:],
                                w1big[:, w_half * KT1 + k, ht * 128 : (ht + 1) * 128],
                                xts[:, k, hs(half)],
                                start=(k == 0),
                                stop=(k == KT1 - 1),
                            )
                    # PSUM -> SBUF copies, split DVE/ACT; b1 folded into ya
                    for m in (0, 2, 1, 3):
                        if m < 2:
                            nc.vector.tensor_scalar_add(
                                y_all[:, m, hs(half)], psA[m][:], bias("b1", m)
                            )
                        else:
                            nc.scalar.copy(y_all[:, m, hs(half)], psA[m][:])

                    # ---- stage B: pair-add + bias on DVE, relu on ACT ----
                    # y_all viewed [128, m, half, c, b]; h1all [128, t, half, p, b]
                    hbase = half * P * BH
                    y5 = y_all.rearrange("p m (hh c b) -> p m hh c b", hh=NH, b=BH)
                    h5 = h1all.rearrange("p t (hh pp b) -> p t hh pp b", hh=NH, b=BH)
                    for i in range(C - 1):
                        nj = C - 1 - i
                        p0 = PAIR_BASE[i]
                        in0 = y5[:, 0:2, half, i : i + 1, :].broadcast_to(
                            [128, 2, nj, BH]
                        )
                        in1 = y5[:, 2:4, half, i + 1 :, :]
                        outap = h5[:, :, half, p0 : p0 + nj, :]
                        nc.vector.tensor_add(outap, in0, in1)
                    # relu in place, both t at once, 960-wide slices (ACT)
                    for sp in range(NSP):
                        sl = h1all[
                            :, :, hbase + sp * 2 * GW : hbase + (sp + 1) * 2 * GW
                        ]
                        nc.scalar.activation(sl, sl, AF.Relu)

                    # flush the PREVIOUS chunk's DVE accumulate chain now, so
                    # this chunk's pair-adds (above) fed PE/ACT first
                    if pend is not None:
                        flush_acc(pend)
                        pend = None

                    # ---- stage C+D: layer-2 matmul, relu(+b2) on ACT ----
                    par = half % 2
                    for sp in range(NSP):
                        for m in range(2):
                            ps = psC_pool.tile(
                                [128, 1024], F32, tag="psC",
                                name=f"psC_{half}_{m}_{sp}",
                            )
                            for sub in range(2):
                                s = sp * 2 + sub
                                for k in range(2):
                                    nc.tensor.matmul(
                                        ps[:, sub * 512 : sub * 512 + GW],
                                        w2t[:, k, m * 128 : (m + 1) * 128],
                                        h1all[
                                            :,
                                            k,
                                            hbase + s * GW : hbase + (s + 1) * GW,
                                        ],
                                        start=(k == 0),
                                        stop=(k == 1),
                                    )
                            h2t = h2sb[m][par][
                                :, sp * 2 * GW : (sp + 1) * 2 * GW
                            ].rearrange("p (u g) -> p u g", g=GW)
                            psv = ps.rearrange("p (u g) -> p u g", g=512)[:, :, :GW]
                            nc.scalar.activation(h2t, psv, AF.Relu, bias=bias("b2", m))
                    pend = (half, par)

                if pend is not None:
                    flush_acc(pend)
                    pend = None

                # ---- stage E: outT = (m2 @ W3scaled) + b3 (bias on DVE) ----
                for mo in range(2):
                    ps = psA_pool.tile([128, HTOK], F32, tag="psA", name=f"psE_{mo}")
                    po = ps[:, :BL]
                    for k in range(2):
                        nc.tensor.matmul(
                            po,
                            w3t[:, k, mo * 128 : (mo + 1) * 128],
                            m2[k][:],
                            start=(k == 0),
                            stop=(k == 1),
                        )
                    nc.vector.tensor_scalar_add(osb[:, mo, :], po, bias("b3", mo))
                nc.sync.dma_start(
                    out=out_d.rearrange("(m p) b -> p m b", p=128), in_=osb[:]
                )

    nc.compile()
    return nc


_NC_CACHE = None


def _get_module():
    global _NC_CACHE
    if _NC_CACHE is None:
        _NC_CACHE = build_module()
    return _NC_CACHE


def make_in_maps(x, W1, b1, W2, b2, W3, b3):
    W1 = np.ascontiguousarray(W1, dtype=np.float32)
    w3p = np.ascontiguousarray(W3, dtype=np.float32) / np.float32(P)
    b1 = np.asarray(b1, dtype=np.float32)
    b2 = np.asarray(b2, dtype=np.float32)
    b3 = np.asarray(b3, dtype=np.float32)
    bias_pack = np.stack(
        [b1[:128], b1[128:], b2[:128], b2[128:], b3[:128], b3[128:]], axis=1
    )
    bias_pack = np.ascontiguousarray(bias_pack, dtype=np.float32)
    in_maps = []
    for i in range(N_CORES):
        xs = x[i * BL : (i + 1) * BL]  # [BL, C, F]
        halves = [
            xs[h * BH : (h + 1) * BH].transpose(1, 0, 2).reshape(HTOK, F)
            for h in range(NH)
        ]
        xT = np.ascontiguousarray(np.concatenate(halves, axis=0).T, dtype=np.float32)
        in_maps.append(
            {
                "xt": xT,
                "w1": W1,
                "w2": np.ascontiguousarray(W2, dtype=np.float32),
                "w3": np.ascontiguousarray(w3p, dtype=np.float32),
                "bias_pack": bias_pack,
                "ident": np.eye(128, dtype=np.float32),
            }
        )
    return in_maps


def kernel(x, W1, b1, W2, b2, W3, b3):
    nc = _get_module()
    in_maps = make_in_maps(
        np.asarray(x, dtype=np.float32),
        np.asarray(W1),
        np.asarray(b1),
        np.asarray(W2),
        np.asarray(b2),
        np.asarray(W3),
        np.asarray(b3),
    )
    res = run_bass_kernel_spmd(nc, in_maps, list(range(N_CORES)))
    out = np.empty((B, H), dtype=np.float32)
    for i in range(N_CORES):
        out[i * BL : (i + 1) * BL] = res.results[i]["outT"].T
    return out



# revision 22
# speedup vs baseline: 1.2217x; 1.2217x over previous
"""Trainium2 Bass kernel for BaseRelationNetwork forward pass.

Reference computation (per batch row b):
    pairs (i<j) of C=16 channels, P=120 pairs
    h1 = relu(concat(x_i, x_j) @ W1 + b1)      # W1 [2F, H]
    h2 = relu(h1 @ W2 + b2)
    out = mean_p(h2 @ W3 + b3)                 # [B, H]

Algebraic restructuring:
  1. W1 splits into W1a/W1b; ya = x @ W1a, yb = x @ W1b computed once per
     channel. h1[p=(i,j)] = relu(ya[i] + yb[j] + b1) via DVE.
  2. mean over pairs commutes with layer 3: out = (mean_p h2) @ W3 + b3.

v2 design:
  - bf16 operands everywhere big (x, W1, W2, identity, y, h1, h2). Halves
    HBM traffic and unlocks DVE 2x (tensor_tensor) / 4x (tensor_scalar)
    modes. PSUM accumulation stays fp32.
  - relu(h1 + b1) is ONE 4x-mode DVE tensor_scalar per (half, t); the
    layer-1 PSUM drain is a plain wide ACT copy.
  - pair-sum of h2: halves 0-2 use a Pool 2:1 add + DVE reduce; half 3
    (the tail) uses PE-identity compaction + short DVE reduce.
  - tiles are per-pipeline-stage (per-chunk x, per-half y/h1, per-(g) psA
    slots) so tile-granular dependency tracking yields the intended
    overlap; W1/W2/W3/biases go on the sync HWDGE queue in parallel with
    the x chunks on the gpsimd SWDGE ring.

Sharding: data-parallel over batch. 512 rows / 8 cores = 64 rows per core,
processed in NH=4 chunks of 16 rows; weights replicated; no collectives.
"""

import contextlib
import sys

if "/opt/trn_rl_repo" not in sys.path:
    sys.path.insert(0, "/opt/trn_rl_repo")

import numpy as np

import concourse.bass as bass
import concourse.mybir as mybir
import concourse.tile as tile
from concourse import bacc
from concourse.bass_utils import run_bass_kernel_spmd

# Problem shape (hardcoded per contract).
B, C, F, H = 512, 16, 1024, 256
N_CORES = 8
BL = B // N_CORES          # 64 local batch rows per core
P = C * (C - 1) // 2       # 120 pairs
NH = 4                     # batch chunks per core
BH = BL // NH              # 16 rows per chunk
TOK = BL * C               # 1024 tokens per core
HTOK = BH * C              # 256 tokens per chunk; token = chunk*256 + c*16 + b
F32 = mybir.dt.float32
BF16 = mybir.dt.bfloat16
BF16_NP = mybir.dt.np(mybir.dt.bfloat16)

KT1 = F // 128             # 8 k-tiles for layer-1 contraction
PPG = 30                   # pairs per stage-C su-block
GW = PPG * BH              # su-block width: 480 columns
NG = P // PPG              # 4 su-blocks per half
NSP = NG // 2              # 2 psC tiles per half (2 su-blocks each)

# per-half pair-sum reducer: "pool2" | "dve" | "pe" (see reduce_half)
REDUCER = {0: "pool2", 1: "pool2", 2: "pe", 3: "pe"}

# pair enumeration: for i in 0..C-2, j in i+1..C-1, p consecutive
PAIR_BASE = [0] * C
for _i in range(1, C):
    PAIR_BASE[_i] = PAIR_BASE[_i - 1] + (C - 1 - (_i - 1))

AF = mybir.ActivationFunctionType
ALU = mybir.AluOpType


def build_module(loop_iters: int = 1):
    nc = bacc.Bacc("TRN2", target_bir_lowering=False, debug=True)

    xt_d = nc.dram_tensor("xt", [F, TOK], BF16, kind="ExternalInput")
    w1_d = nc.dram_tensor("w1", [2 * F, H], BF16, kind="ExternalInput")
    w2_d = nc.dram_tensor("w2", [H, H], BF16, kind="ExternalInput")
    w3_d = nc.dram_tensor("w3", [H, H], F32, kind="ExternalInput")
    bp_d = nc.dram_tensor("bias_pack", [128, 6], F32, kind="ExternalInput")
    id_d = nc.dram_tensor("ident", [128, 128], BF16, kind="ExternalInput")
    out_d = nc.dram_tensor("outT", [H, BL], F32, kind="ExternalOutput")

    with tile.TileContext(nc) as tc:
        with (
            tc.tile_pool(name="xpool", bufs=1) as xpool,
            tc.tile_pool(name="wpool", bufs=1) as wpool,
            tc.tile_pool(name="ypool", bufs=1) as ypool,
            tc.tile_pool(name="hpool", bufs=1) as hpool,
            tc.tile_pool(name="spool", bufs=1) as spool,
            tc.tile_pool(name="psA", bufs=1, space="PSUM") as psA_pool,
            tc.tile_pool(name="psC", bufs=2, space="PSUM") as psC_pool,
            tc.tile_pool(name="psR", bufs=2, space="PSUM") as psR_pool,
        ):
            loop_cm = (
                tc.For_i(0, loop_iters, 1)
                if loop_iters > 1
                else contextlib.nullcontext()
            )
            with loop_cm:
                # x chunk tiles
                xch = [
                    xpool.tile([128, KT1, HTOK], BF16, tag=f"x{h}", name=f"x{h}")
                    for h in range(NH)
                ]
                w1a = wpool.tile([128, KT1, H], BF16, tag="w1a", name="w1a")
                w1b = wpool.tile([128, KT1, H], BF16, tag="w1b", name="w1b")
                w2t = wpool.tile([128, 2, H], BF16, tag="w2t", name="w2t")
                w3t = wpool.tile([128, 2, H], F32, tag="w3t", name="w3t")
                bp = wpool.tile([128, 6], F32, tag="bp", name="bp")
                idt = wpool.tile([128, 128], BF16, tag="idt", name="idt")
                wsrc = wpool.tile([128, 16], BF16, tag="wsrc", name="wsrc")
                # per-half y tiles, free layout [m(4), c(C), b(BH)]
                yh = [
                    ypool.tile([128, 4, HTOK], BF16, tag=f"y{h}", name=f"y{h}")
                    for h in range(NH)
                ]
                # per-half h1 tiles, free layout [t(2), p(P), b(BH)]
                h1h = [
                    hpool.tile([128, 2, P * BH], BF16, tag=f"h1_{h}", name=f"h1_{h}")
                    for h in range(NH)
                ]
                # h2 tiles split per sp (su-block pair) for finer deps
                h2sb = [
                    [
                        [
                            spool.tile(
                                [128, 2 * GW], BF16,
                                tag=f"h2_{m}_{par}_{sp}",
                                name=f"h2_{m}_{par}_{sp}",
                            )
                            for sp in range(NSP)
                        ]
                        for par in range(2)
                    ]
                    for m in range(2)
                ]
                # scratch for the Pool 2:1 tree level, per (m, parity)
                tr2 = [
                    [
                        spool.tile(
                            [128, 2 * GW], BF16,
                            tag=f"tr2_{m}_{par}", name=f"tr2_{m}_{par}",
                        )
                        for par in range(2)
                    ]
                    for m in range(2)
                ]
                m2 = [
                    spool.tile([128, BL], F32, tag=f"m2_{m}", name=f"m2_{m}")
                    for m in range(2)
                ]
                osb = spool.tile([128, 2, BL], F32, tag="osb", name="osb")

                def bias(nm, t):
                    idx = {"b1": 0, "b2": 2, "b3": 4}[nm] + t
                    return bp[:, idx : idx + 1]

                # DRAM views
                w1v = w1_d.rearrange("(k p) h -> p k h", p=128)
                xtv = xt_d.rearrange("(k p) t -> p k t", p=128)

                def hs(half):
                    return slice(half * HTOK, (half + 1) * HTOK)

                # ---- DMA: weights/smalls on sync HWDGE; x on SWDGE ring.
                # All transfers serialize on one DMA channel, so the order
                # here is the transfer schedule: w1a first, x0 next (stage-A
                # critical path), then alternate weights with x chunks. ----
                nc.sync.dma_start(out=w1a[:], in_=w1v[:, 0:KT1, :])
                nc.sync.dma_start(out=bp[:], in_=bp_d[:])
                nc.sync.dma_start(out=w1b[:], in_=w1v[:, KT1 : 2 * KT1, :])
                nc.sync.dma_start(
                    out=w2t[:], in_=w2_d.rearrange("(k p) h -> p k h", p=128)
                )
                nc.sync.dma_start(out=idt[:], in_=id_d[:])
                nc.sync.dma_start(
                    out=w3t[:], in_=w3_d.rearrange("(k p) h -> p k h", p=128)
                )
                for ch in range(NH):
                    nc.gpsimd.dma_start(out=xch[ch][:], in_=xtv[:, :, hs(ch)])

                # PE warm-up from t~0: memset a small bf16 source, then
                # dummy matmuls to ramp the PE clock while DMAs stream
                nc.vector.memset(wsrc[:], 0)
                warm = psC_pool.tile([128, 1024], F32, tag="psC", name="warm")
                for _ in range(7):
                    nc.tensor.matmul(
                        warm[:1, :512],
                        wsrc[:, 0:1],
                        wsrc[:, 0:1].broadcast_to([128, 512]),
                        start=True,
                        stop=True,
                    )

                def stage_A_piece(half, g, mt, psg):
                    """One layer-1 accumulation chain (8 matmuls)."""
                    w1g = w1a if g == 0 else w1b
                    for k in range(KT1):
                        nc.tensor.matmul(
                            psg[:, mt, :],
                            w1g[:, k, mt * 128 : (mt + 1) * 128],
                            xch[half][:, k, :],
                            start=(k == 0),
                            stop=(k == KT1 - 1),
                        )

                def stage_A_pieces(half):
                    """Layer-1 work for chunk `half` as 4 schedulable pieces
                    (2 chains per g; ACT drain after each g's second chain)."""
                    pieces = []
                    for g in range(2):
                        psg = psA_pool.tile(
                            [128, 2, HTOK], F32, tag=f"psA{g}",
                            name=f"psA{g}_{half}",
                        )

                        def mk(g=g, psg=psg, mt=0):
                            stage_A_piece(half, g, mt, psg)

                        def mk_last(g=g, psg=psg):
                            stage_A_piece(half, g, 1, psg)
                            nc.scalar.copy(
                                yh[half][:, 2 * g : 2 * g + 2, :], psg[:]
                            )

                        pieces.append(mk)
                        pieces.append(mk_last)
                    return pieces

                def stage_A(half):
                    for piece in stage_A_pieces(half):
                        piece()

                def stage_B(half):
                    """Pair-add on DVE (bf16 2x), then fused relu(h1+b1) via
                    4x-mode tensor_scalar per t."""
                    y5 = yh[half].rearrange("p m (c b) -> p m c b", b=BH)
                    h5 = h1h[half].rearrange("p t (pp b) -> p t pp b", b=BH)
                    for i in range(C - 1):
                        nj = C - 1 - i
                        p0 = PAIR_BASE[i]
                        in0 = y5[:, 0:2, i : i + 1, :].broadcast_to(
                            [128, 2, nj, BH]
                        )
                        in1 = y5[:, 2:4, i + 1 :, :]
                        outap = h5[:, :, p0 : p0 + nj, :]
                        nc.vector.tensor_add(outap, in0, in1)
                    for t in range(2):
                        sl = h1h[half][:, t, :]
                        nc.vector.tensor_scalar(
                            sl, sl, bias("b1", t), 0.0, ALU.add, ALU.max
                        )

                def stage_C(half, fillers=()):
                    """Layer-2 matmuls + ACT relu(+b2) drain into h2sb.
                    After each psC tile's matmuls, one pending filler (extra
                    PE work) is emitted to cover the ACT drain latency."""
                    fillers = list(fillers)
                    par = half % 2
                    for sp in range(NSP):
                        for m in range(2):
                            ps = psC_pool.tile(
                                [128, 1024], F32, tag="psC",
                                name=f"psC_{half}_{m}_{sp}",
                            )
                            for sub in range(2):
                                s = sp * 2 + sub
                                for k in range(2):
                                    nc.tensor.matmul(
                                        ps[:, sub * 512 : sub * 512 + GW],
                                        w2t[:, k, m * 128 : (m + 1) * 128],
                                        h1h[half][
                                            :, k, s * GW : (s + 1) * GW
                                        ],
                                        start=(k == 0),
                                        stop=(k == 1),
                                    )
                            h2t = h2sb[m][par][sp][:, :].rearrange(
                                "p (u g) -> p u g", g=GW
                            )
                            psv = ps.rearrange("p (u g) -> p u g", g=512)[:, :, :GW]
                            nc.scalar.activation(h2t, psv, AF.Relu, bias=bias("b2", m))
                            if fillers:
                                fillers.pop(0)()
                    for f in fillers:
                        f()

                def reduce_unit(half, m):
                    """Pair-sum of h2 for one (half, m) -> m2 slice."""
                    par = half % 2
                    kind = REDUCER[half]
                    if True:
                        dst = m2[m][:, half * BH : (half + 1) * BH]
                        if kind == "pool2":
                            t = tr2[m][par]
                            nc.gpsimd.tensor_add(
                                t[:], h2sb[m][par][0][:], h2sb[m][par][1][:]
                            )
                            v = (
                                t.rearrange("q (pp b) -> q pp b", b=BH)
                                .transpose([0, 2, 1])
                            )
                            nc.vector.tensor_reduce(
                                dst, v, mybir.AxisListType.X, ALU.add
                            )
                        else:  # pe: identity compaction + short DVE reduce
                            psr = psR_pool.tile(
                                [128, 512], F32, tag="psR", name=f"psR_{half}_{m}"
                            )
                            for su in range(NG):
                                nc.tensor.matmul(
                                    psr[:, :GW],
                                    idt[:],
                                    h2sb[m][par][su // 2][
                                        :, (su % 2) * GW : (su % 2 + 1) * GW
                                    ],
                                    start=(su == 0),
                                    stop=(su == NG - 1),
                                )
                            v = (
                                psr[:, :GW]
                                .rearrange("q (pp b) -> q pp b", b=BH)
                                .transpose([0, 2, 1])
                            )
                            nc.vector.tensor_reduce(
                                dst, v, mybir.AxisListType.X, ALU.add
                            )

                def reduce_half(half):
                    for m in range(2):
                        reduce_unit(half, m)

                # ---- pipeline: PE order A0 A1 A2 C0[A3] C1 C2 C3[R2] R3 E;
                # A3 fills C0's drain gaps, comp(2) fills C3's ----
                stage_A(0)
                stage_A(1)
                stage_B(0)
                stage_A(2)
                stage_B(1)
                stage_C(0, fillers=stage_A_pieces(3))
                stage_B(2)
                stage_B(3)
                stage_C(1)
                reduce_half(0)
                stage_C(2)
                reduce_half(1)
                stage_C(
                    3,
                    fillers=[
                        lambda: reduce_unit(2, 0),
                        lambda: reduce_unit(2, 1),
                    ],
                )
                reduce_half(3)

                # ---- stage E: outT = (m2 @ W3scaled) + b3 (bias on DVE) ----
                for mo in range(2):
                    ps = psR_pool.tile([128, 512], F32, tag="psR", name=f"psE_{mo}")
                    po = ps[:, :BL]
                    for k in range(2):
                        nc.tensor.matmul(
                            po,
                            w3t[:, k, mo * 128 : (mo + 1) * 128],
                            m2[k][:],
                            start=(k == 0),
                            stop=(k == 1),
                        )
                    nc.vector.tensor_scalar_add(osb[:, mo, :], po, bias("b3", mo))
                nc.sync.dma_start(
                    out=out_d.rearrange("(m p) b -> p m b", p=128), in_=osb[:]
                )

    nc.compile()
    return nc


_NC_CACHE = None


def _get_module():
    global _NC_CACHE
    if _NC_CACHE is None:
        _NC_CACHE = build_module()
    return _NC_CACHE


def make_in_maps(x, W1, b1, W2, b2, W3, b3):
    W1 = np.ascontiguousarray(np.asarray(W1, dtype=np.float32).astype(BF16_NP))
    W2 = np.ascontiguousarray(np.asarray(W2, dtype=np.float32).astype(BF16_NP))
    w3p = np.ascontiguousarray(W3, dtype=np.float32) / np.float32(P)
    b1 = np.asarray(b1, dtype=np.float32)
    b2 = np.asarray(b2, dtype=np.float32)
    b3 = np.asarray(b3, dtype=np.float32)
    bias_pack = np.stack(
        [b1[:128], b1[128:], b2[:128], b2[128:], b3[:128], b3[128:]], axis=1
    )
    bias_pack = np.ascontiguousarray(bias_pack, dtype=np.float32)
    ident = np.eye(128, dtype=np.float32).astype(BF16_NP)
    in_maps = []
    for i in range(N_CORES):
        xs = x[i * BL : (i + 1) * BL]  # [BL, C, F]
        halves = [
            xs[h * BH : (h + 1) * BH].transpose(1, 0, 2).reshape(HTOK, F)
            for h in range(NH)
        ]
        xT = np.ascontiguousarray(
            np.concatenate(halves, axis=0).T.astype(BF16_NP)
        )
        in_maps.append(
            {
                "xt": xT,
                "w1": W1,
                "w2": W2,
                "w3": np.ascontiguousarray(w3p, dtype=np.float32),
                "bias_pack": bias_pack,
                "ident": ident,
            }
        )
    return in_maps


def kernel(x, W1, b1, W2, b2, W3, b3):
    nc = _get_module()
    in_maps = make_in_maps(
        np.asarray(x, dtype=np.float32),
        np.asarray(W1),
        np.asarray(b1),
        np.asarray(W2),
        np.asarray(b2),
        np.asarray(W3),
        np.asarray(b3),
    )
    res = run_bass_kernel_spmd(nc, in_maps, list(range(N_CORES)))
    out = np.empty((B, H), dtype=np.float32)
    for i in range(N_CORES):
        out[i * BL : (i + 1) * BL] = res.results[i]["outT"].T
    return out


# revision 40
# speedup vs baseline: 1.2978x; 1.0622x over previous
"""Trainium2 Bass kernel for BaseRelationNetwork forward pass.

Reference computation (per batch row b):
    pairs (i<j) of C=16 channels, P=120 pairs
    h1 = relu(concat(x_i, x_j) @ W1 + b1)      # W1 [2F, H]
    h2 = relu(h1 @ W2 + b2)
    out = mean_p(h2 @ W3 + b3)                 # [B, H]

Algebraic restructuring:
  1. W1 splits into W1a/W1b; ya = x @ W1a, yb = x @ W1b computed once per
     channel. h1[p=(i,j)] = relu(ya[i] + yb[j] + b1) via DVE.
  2. mean over pairs commutes with layer 3: out = (mean_p h2) @ W3 + b3.

v2 design (vs the fp32r baseline: 73us -> ~54us per iteration):
  - bf16 operands everywhere big (x, W1, W2, identity, y, h1, h2). Halves
    HBM traffic and unlocks the DVE 2x (tensor_tensor) / 4x (tensor_scalar)
    fast modes. PSUM accumulation stays fp32; rel err ~9e-4.
  - relu(h1 + b1) is ONE 4x-mode DVE tensor_scalar per (half, t) (the
    same op on ACT costs +9us/iter); the layer-1 PSUM drain is a plain
    wide ACT copy. Layer-1 chains run to completion per m-tile - two
    interleaved accumulation groups in one PSUM bank corrupt results.
  - pair-sum of h2: halves 0/1 use a Pool 2:1 add + DVE reduce; halves
    2/3 use PE-identity compaction + short DVE reduce, with half 2's
    compaction interleaved into stage_C(3)'s drain gaps (one full phase
    of slack - eager fillers stall PE's in-order queue).
  - tiles are per-pipeline-stage (per-chunk x, per-half y/h1, per-(g) psA
    slots) so tile-granular dependency tracking yields the intended
    overlap; W1/W2/W3/biases go on the sync HWDGE queue, x chunks on the
    gpsimd SWDGE ring (all transfers serialize on one DMA channel, so
    emission order w1a, x0, w1b, x1.. is the schedule).
  - the timing loop uses For_i(staggered_reset=True): the default
    all-engine barrier per iteration serializes the phases and costs
    ~6us/iter.

The K_* env vars below are experiment knobs; the defaults are the tuned
configuration and are what the graded kernel() path uses.

Sharding: data-parallel over batch. 512 rows / 8 cores = 64 rows per core,
processed in NH=4 chunks of 16 rows; weights replicated; no collectives.
"""

import contextlib
import sys

if "/opt/trn_rl_repo" not in sys.path:
    sys.path.insert(0, "/opt/trn_rl_repo")

import numpy as np

import concourse.bass as bass
import concourse.mybir as mybir
import concourse.tile as tile
from concourse import bacc
from concourse.bass_utils import run_bass_kernel_spmd

# Problem shape (hardcoded per contract).
B, C, F, H = 512, 16, 1024, 256
N_CORES = 8
BL = B // N_CORES          # 64 local batch rows per core
P = C * (C - 1) // 2       # 120 pairs
NH = 4                     # batch chunks per core
BH = BL // NH              # 16 rows per chunk
TOK = BL * C               # 1024 tokens per core
HTOK = BH * C              # 256 tokens per chunk; token = chunk*256 + c*16 + b
F32 = mybir.dt.float32
BF16 = mybir.dt.bfloat16
BF16_NP = mybir.dt.np(mybir.dt.bfloat16)

KT1 = F // 128             # 8 k-tiles for layer-1 contraction
PPG = 30                   # pairs per stage-C su-block
GW = PPG * BH              # su-block width: 480 columns
NG = P // PPG              # 4 su-blocks per half
NSP = NG // 2              # 2 psC tiles per half (2 su-blocks each)

import os


def _cfg():
    red = os.environ.get("K_REDUCERS", "pool2,pool2,pe,pe").replace(
        "+", ","
    ).split(",")
    return {
        "reducer": {h: red[h] for h in range(4)},
        "n_warm": int(os.environ.get("K_WARM", "7")),
        "relu_on_act": os.environ.get("K_RELU_ACT", "0") == "1",
        "outdma": os.environ.get("K_OUTDMA", "sync"),
        # subtractive profiling: full | noE | noC | AB | A | dma
        "cut": os.environ.get("K_CUT", "full"),
        # number of trailing halves whose m=1 h2-drains go to DVE
        "dvedrain": int(os.environ.get("K_DVEDRAIN", "0")),
        # split the out-DMA into one transfer per E m-tile
        "splitout": os.environ.get("K_SPLITOUT", "0") == "1",
        # explicit staggered-reset stage boundaries at phase edges
        "sbound": os.environ.get("K_SBOUND", "0") == "1",
    }

# pair enumeration: for i in 0..C-2, j in i+1..C-1, p consecutive
PAIR_BASE = [0] * C
for _i in range(1, C):
    PAIR_BASE[_i] = PAIR_BASE[_i - 1] + (C - 1 - (_i - 1))

AF = mybir.ActivationFunctionType
ALU = mybir.AluOpType


def build_module(loop_iters: int = 1):
    cfg = _cfg()
    REDUCER = cfg["reducer"]
    N_WARM = cfg["n_warm"]
    RELU_ON_ACT = cfg["relu_on_act"]
    nc = bacc.Bacc("TRN2", target_bir_lowering=False, debug=True)

    xt_d = nc.dram_tensor("xt", [F, TOK], BF16, kind="ExternalInput")
    w1_d = nc.dram_tensor("w1", [2 * F, H], BF16, kind="ExternalInput")
    w2_d = nc.dram_tensor("w2", [H, H], BF16, kind="ExternalInput")
    w3_d = nc.dram_tensor("w3", [H, H], F32, kind="ExternalInput")
    bp_d = nc.dram_tensor("bias_pack", [128, 6], F32, kind="ExternalInput")
    id_d = nc.dram_tensor("ident", [128, 128], BF16, kind="ExternalInput")
    out_d = nc.dram_tensor("outT", [H, BL], F32, kind="ExternalOutput")

    with tile.TileContext(nc) as tc:
        with (
            tc.tile_pool(name="xpool", bufs=1) as xpool,
            tc.tile_pool(name="wpool", bufs=1) as wpool,
            tc.tile_pool(name="ypool", bufs=1) as ypool,
            tc.tile_pool(name="hpool", bufs=1) as hpool,
            tc.tile_pool(name="spool", bufs=1) as spool,
            tc.tile_pool(name="psA", bufs=1, space="PSUM") as psA_pool,
            tc.tile_pool(name="psC", bufs=2, space="PSUM") as psC_pool,
            tc.tile_pool(name="psR", bufs=2, space="PSUM") as psR_pool,
        ):
            loop_cm = (
                tc.For_i(
                    0, loop_iters, 1,
                    staggered_reset=os.environ.get("K_STAGGER", "1") == "1",
                )
                if loop_iters > 1
                else contextlib.nullcontext()
            )
            with loop_cm:
                # x chunk tiles
                xch = [
                    xpool.tile([128, KT1, HTOK], BF16, tag=f"x{h}", name=f"x{h}")
                    for h in range(NH)
                ]
                w1a = wpool.tile([128, KT1, H], BF16, tag="w1a", name="w1a")
                w1b = wpool.tile([128, KT1, H], BF16, tag="w1b", name="w1b")
                w2t = wpool.tile([128, 2, H], BF16, tag="w2t", name="w2t")
                w3t = wpool.tile([128, 2, H], F32, tag="w3t", name="w3t")
                bp = wpool.tile([128, 6], F32, tag="bp", name="bp")
                idt = wpool.tile([128, 128], BF16, tag="idt", name="idt")
                wsrc = wpool.tile([128, 16], BF16, tag="wsrc", name="wsrc")
                # per-half y tiles, free layout [m(4), c(C), b(BH)]
                yh = [
                    ypool.tile([128, 4, HTOK], BF16, tag=f"y{h}", name=f"y{h}")
                    for h in range(NH)
                ]
                # per-half h1 tiles, free layout [t(2), p(P), b(BH)]
                h1h = [
                    hpool.tile([128, 2, P * BH], BF16, tag=f"h1_{h}", name=f"h1_{h}")
                    for h in range(NH)
                ]
                # h2 tiles split per sp (su-block pair) for finer deps
                h2sb = [
                    [
                        [
                            spool.tile(
                                [128, 2 * GW], BF16,
                                tag=f"h2_{m}_{par}_{sp}",
                                name=f"h2_{m}_{par}_{sp}",
                            )
                            for sp in range(NSP)
                        ]
                        for par in range(2)
                    ]
                    for m in range(2)
                ]
                # scratch for the Pool 2:1 tree level, per (m, parity)
                tr2 = [
                    [
                        spool.tile(
                            [128, 2 * GW], BF16,
                            tag=f"tr2_{m}_{par}", name=f"tr2_{m}_{par}",
                        )
                        for par in range(2)
                    ]
                    for m in range(2)
                ]
                m2 = [
                    spool.tile([128, BL], F32, tag=f"m2_{m}", name=f"m2_{m}")
                    for m in range(2)
                ]
                osb = spool.tile([128, 2, BL], F32, tag="osb", name="osb")

                def bias(nm, t):
                    idx = {"b1": 0, "b2": 2, "b3": 4}[nm] + t
                    return bp[:, idx : idx + 1]

                # DRAM views
                w1v = w1_d.rearrange("(k p) h -> p k h", p=128)
                xtv = xt_d.rearrange("(k p) t -> p k t", p=128)

                def hs(half):
                    return slice(half * HTOK, (half + 1) * HTOK)

                # ---- DMA: weights/smalls on sync HWDGE; x on SWDGE ring.
                # All transfers serialize on one DMA channel, so the order
                # here is the transfer schedule: w1a first, x0 next (stage-A
                # critical path), then alternate weights with x chunks. ----
                nc.sync.dma_start(out=w1a[:], in_=w1v[:, 0:KT1, :])
                nc.sync.dma_start(out=bp[:], in_=bp_d[:])
                nc.sync.dma_start(out=w1b[:], in_=w1v[:, KT1 : 2 * KT1, :])
                nc.sync.dma_start(
                    out=w2t[:], in_=w2_d.rearrange("(k p) h -> p k h", p=128)
                )
                nc.sync.dma_start(out=idt[:], in_=id_d[:])
                nc.sync.dma_start(
                    out=w3t[:], in_=w3_d.rearrange("(k p) h -> p k h", p=128)
                )
                for ch in range(NH):
                    nc.gpsimd.dma_start(out=xch[ch][:], in_=xtv[:, :, hs(ch)])

                # PE warm-up from t~0: memset a small bf16 source, then
                # dummy matmuls to ramp the PE clock while DMAs stream.
                # Kept short: in the timing loop PE stays hot across
                # iterations and these are pure overhead.
                nc.vector.memset(wsrc[:], 0)
                warm = psC_pool.tile([128, 1024], F32, tag="psC", name="warm")
                for _ in range(N_WARM):
                    nc.tensor.matmul(
                        warm[:1, :512],
                        wsrc[:, 0:1],
                        wsrc[:, 0:1].broadcast_to([128, 512]),
                        start=True,
                        stop=True,
                    )

                def stage_A_piece(half, g, mt, psg):
                    """One layer-1 accumulation chain (8 matmuls)."""
                    w1g = w1a if g == 0 else w1b
                    for k in range(KT1):
                        nc.tensor.matmul(
                            psg[:, mt, :],
                            w1g[:, k, mt * 128 : (mt + 1) * 128],
                            xch[half][:, k, :],
                            start=(k == 0),
                            stop=(k == KT1 - 1),
                        )

                def stage_A_pieces(half):
                    """Layer-1 work for chunk `half` as 4 schedulable pieces
                    (2 chains per g; ACT drain after each g's second chain)."""
                    pieces = []
                    for g in range(2):
                        psg = psA_pool.tile(
                            [128, 2, HTOK], F32, tag=f"psA{g}",
                            name=f"psA{g}_{half}",
                        )

                        def mk(g=g, psg=psg, mt=0):
                            stage_A_piece(half, g, mt, psg)

                        def mk_last(g=g, psg=psg):
                            stage_A_piece(half, g, 1, psg)
                            nc.scalar.copy(
                                yh[half][:, 2 * g : 2 * g + 2, :], psg[:]
                            )

                        pieces.append(mk)
                        pieces.append(mk_last)
                    return pieces

                def stage_A(half):
                    for piece in stage_A_pieces(half):
                        piece()

                def stage_B(half):
                    """Pair-add on DVE (bf16 2x), then fused relu(h1+b1) via
                    4x-mode tensor_scalar per t."""
                    y5 = yh[half].rearrange("p m (c b) -> p m c b", b=BH)
                    h5 = h1h[half].rearrange("p t (pp b) -> p t pp b", b=BH)
                    for i in range(C - 1):
                        nj = C - 1 - i
                        p0 = PAIR_BASE[i]
                        in0 = y5[:, 0:2, i : i + 1, :].broadcast_to(
                            [128, 2, nj, BH]
                        )
                        in1 = y5[:, 2:4, i + 1 :, :]
                        outap = h5[:, :, p0 : p0 + nj, :]
                        nc.vector.tensor_add(outap, in0, in1)
                    for t in range(2):
                        sl = h1h[half][:, t, :]
                        if RELU_ON_ACT:
                            nc.scalar.activation(
                                sl, sl, AF.Relu, bias=bias("b1", t)
                            )
                        else:
                            nc.vector.tensor_scalar(
                                sl, sl, bias("b1", t), 0.0, ALU.add, ALU.max
                            )

                def stage_C(half, fillers=()):
                    """Layer-2 matmuls + ACT relu(+b2) drain into h2sb.
                    After each psC tile's matmuls, one pending filler (extra
                    PE work) is emitted to cover the ACT drain latency."""
                    fillers = list(fillers)
                    par = half % 2
                    for sp in range(NSP):
                        for m in range(2):
                            ps = psC_pool.tile(
                                [128, 1024], F32, tag="psC",
                                name=f"psC_{half}_{m}_{sp}",
                            )
                            for sub in range(2):
                                s = sp * 2 + sub
                                for k in range(2):
                                    nc.tensor.matmul(
                                        ps[:, sub * 512 : sub * 512 + GW],
                                        w2t[:, k, m * 128 : (m + 1) * 128],
                                        h1h[half][
                                            :, k, s * GW : (s + 1) * GW
                                        ],
                                        start=(k == 0),
                                        stop=(k == 1),
                                    )
                            h2t = h2sb[m][par][sp][:, :].rearrange(
                                "p (u g) -> p u g", g=GW
                            )
                            psv = ps.rearrange("p (u g) -> p u g", g=512)[:, :, :GW]
                            if m == 1 and half >= NH - cfg["dvedrain"]:
                                nc.vector.tensor_scalar(
                                    h2t, psv, bias("b2", m), 0.0,
                                    ALU.add, ALU.max,
                                )
                            else:
                                nc.scalar.activation(
                                    h2t, psv, AF.Relu, bias=bias("b2", m)
                                )
                            if fillers:
                                fillers.pop(0)()
                    for f in fillers:
                        f()

                def reduce_unit(half, m):
                    """Pair-sum of h2 for one (half, m) -> m2 slice."""
                    par = half % 2
                    kind = REDUCER[half]
                    if True:
                        dst = m2[m][:, half * BH : (half + 1) * BH]
                        if kind == "pool2":
                            t = tr2[m][par]
                            nc.gpsimd.tensor_add(
                                t[:], h2sb[m][par][0][:], h2sb[m][par][1][:]
                            )
                            v = (
                                t.rearrange("q (pp b) -> q pp b", b=BH)
                                .transpose([0, 2, 1])
                            )
                            nc.vector.tensor_reduce(
                                dst, v, mybir.AxisListType.X, ALU.add
                            )
                        else:  # pe: identity compaction + short DVE reduce
                            psr = psR_pool.tile(
                                [128, 512], F32, tag="psR", name=f"psR_{half}_{m}"
                            )
                            for su in range(NG):
                                nc.tensor.matmul(
                                    psr[:, :GW],
                                    idt[:],
                                    h2sb[m][par][su // 2][
                                        :, (su % 2) * GW : (su % 2 + 1) * GW
                                    ],
                                    start=(su == 0),
                                    stop=(su == NG - 1),
                                )
                            v = (
                                psr[:, :GW]
                                .rearrange("q (pp b) -> q pp b", b=BH)
                                .transpose([0, 2, 1])
                            )
                            nc.vector.tensor_reduce(
                                dst, v, mybir.AxisListType.X, ALU.add
                            )

                def reduce_half(half):
                    for m in range(2):
                        reduce_unit(half, m)

                # ---- pipeline: PE order A0 A1 A2 C0[A3] C1 C2 C3[R2] R3 E;
                # A3 fills C0's drain gaps, comp(2) fills C3's ----
                cut = cfg["cut"]

                def pe_fillers(h):
                    # interleave half h's PE-identity reduce units into the
                    # following C phase's drain gaps
                    if h >= 0 and REDUCER[h] == "pe":
                        return [
                            lambda: reduce_unit(h, 0),
                            lambda: reduce_unit(h, 1),
                        ]
                    return []

                sbound = (
                    cfg["sbound"] and loop_iters > 1
                    and os.environ.get("K_STAGGER", "0") == "1"
                )

                def sb():
                    if sbound:
                        tc.stage_boundary()

                if cut in ("full", "noE", "noC", "AB", "A"):
                    stage_A(0)
                    stage_A(1)
                    sb()
                    if cut != "A":
                        stage_B(0)
                    stage_A(2)
                    if cut != "A":
                        stage_B(1)
                    if cut in ("full", "noE"):
                        stage_C(0, fillers=stage_A_pieces(3))
                    else:
                        stage_A(3)
                    sb()
                    if cut != "A":
                        stage_B(2)
                        stage_B(3)
                    if cut in ("full", "noE"):
                        # pe-reduce fillers for half h may only interleave
                        # into stage_C(h+1): same-parity C(h+2) overwrites
                        # the h2 tiles that comp(h) reads
                        stage_C(1, fillers=pe_fillers(0))
                        if REDUCER[0] != "pe":
                            reduce_half(0)
                        sb()
                        stage_C(2, fillers=pe_fillers(1))
                        if REDUCER[1] != "pe":
                            reduce_half(1)
                        stage_C(3, fillers=pe_fillers(2))
                        if REDUCER[2] != "pe":
                            reduce_half(2)
                        reduce_half(3)
                    else:
                        sb()
                else:
                    sb()
                    sb()
                    sb()

                # ---- stage E: outT = (m2 @ W3scaled) + b3 (bias on DVE) ----
                if cut != "full":
                    nc.vector.memset(osb[:], 0)
                outeng = nc.sync if cfg["outdma"] == "sync" else nc.scalar
                odv = out_d.rearrange("(m p) b -> p m b", p=128)
                for mo in range(2) if cut == "full" else []:
                    ps = psR_pool.tile([128, 512], F32, tag="psR", name=f"psE_{mo}")
                    po = ps[:, :BL]
                    for k in range(2):
                        nc.tensor.matmul(
                            po,
                            w3t[:, k, mo * 128 : (mo + 1) * 128],
                            m2[k][:],
                            start=(k == 0),
                            stop=(k == 1),
                        )
                    nc.vector.tensor_scalar_add(osb[:, mo, :], po, bias("b3", mo))
                    if cfg["splitout"]:
                        outeng.dma_start(
                            out=odv[:, mo, :], in_=osb[:, mo, :]
                        )
                if cut != "full" or not cfg["splitout"]:
                    outeng.dma_start(out=odv[:], in_=osb[:])

    nc.compile()
    return nc


_NC_CACHE = None


def _get_module():
    global _NC_CACHE
    if _NC_CACHE is None:
        _NC_CACHE = build_module()
    return _NC_CACHE


def make_in_maps(x, W1, b1, W2, b2, W3, b3):
    W1 = np.ascontiguousarray(np.asarray(W1, dtype=np.float32).astype(BF16_NP))
    W2 = np.ascontiguousarray(np.asarray(W2, dtype=np.float32).astype(BF16_NP))
    w3p = np.ascontiguousarray(W3, dtype=np.float32) / np.float32(P)
    b1 = np.asarray(b1, dtype=np.float32)
    b2 = np.asarray(b2, dtype=np.float32)
    b3 = np.asarray(b3, dtype=np.float32)
    bias_pack = np.stack(
        [b1[:128], b1[128:], b2[:128], b2[128:], b3[:128], b3[128:]], axis=1
    )
    bias_pack = np.ascontiguousarray(bias_pack, dtype=np.float32)
    ident = np.eye(128, dtype=np.float32).astype(BF16_NP)
    in_maps = []
    for i in range(N_CORES):
        xs = x[i * BL : (i + 1) * BL]  # [BL, C, F]
        halves = [
            xs[h * BH : (h + 1) * BH].transpose(1, 0, 2).reshape(HTOK, F)
            for h in range(NH)
        ]
        xT = np.ascontiguousarray(
            np.concatenate(halves, axis=0).T.astype(BF16_NP)
        )
        in_maps.append(
            {
                "xt": xT,
                "w1": W1,
                "w2": W2,
                "w3": np.ascontiguousarray(w3p, dtype=np.float32),
                "bias_pack": bias_pack,
                "ident": ident,
            }
        )
    return in_maps


def kernel(x, W1, b1, W2, b2, W3, b3):
    nc = _get_module()
    in_maps = make_in_maps(
        np.asarray(x, dtype=np.float32),
        np.asarray(W1),
        np.asarray(b1),
        np.asarray(W2),
        np.asarray(b2),
        np.asarray(W3),
        np.asarray(b3),
    )
    res = run_bass_kernel_spmd(nc, in_maps, list(range(N_CORES)))
    out = np.empty((B, H), dtype=np.float32)
    for i in range(N_CORES):
        out[i * BL : (i + 1) * BL] = res.results[i]["outT"].T
    return out


# revision 41
# speedup vs baseline: 1.3020x; 1.0033x over previous
"""Trainium2 Bass kernel for BaseRelationNetwork forward pass.

Reference computation (per batch row b):
    pairs (i<j) of C=16 channels, P=120 pairs
    h1 = relu(concat(x_i, x_j) @ W1 + b1)      # W1 [2F, H]
    h2 = relu(h1 @ W2 + b2)
    out = mean_p(h2 @ W3 + b3)                 # [B, H]

Algebraic restructuring:
  1. W1 splits into W1a/W1b; ya = x @ W1a, yb = x @ W1b computed once per
     channel. h1[p=(i,j)] = relu(ya[i] + yb[j] + b1) via DVE.
  2. mean over pairs commutes with layer 3: out = (mean_p h2) @ W3 + b3.

v2 design (vs the fp32r baseline: 73us -> ~54us per iteration):
  - bf16 operands everywhere big (x, W1, W2, identity, y, h1, h2). Halves
    HBM traffic and unlocks the DVE 2x (tensor_tensor) / 4x (tensor_scalar)
    fast modes. PSUM accumulation stays fp32; rel err ~9e-4.
  - relu(h1 + b1) is ONE 4x-mode DVE tensor_scalar per (half, t) (the
    same op on ACT costs +9us/iter); the layer-1 PSUM drain is a plain
    wide ACT copy. Layer-1 chains run to completion per m-tile - two
    interleaved accumulation groups in one PSUM bank corrupt results.
  - pair-sum of h2: halves 0/1 use a Pool 2:1 add + DVE reduce; halves
    2/3 use PE-identity compaction + short DVE reduce, with half 2's
    compaction interleaved into stage_C(3)'s drain gaps (one full phase
    of slack - eager fillers stall PE's in-order queue).
  - tiles are per-pipeline-stage (per-chunk x, per-half y/h1, per-(g) psA
    slots) so tile-granular dependency tracking yields the intended
    overlap; W1/W2/W3/biases go on the sync HWDGE queue, x chunks on the
    gpsimd SWDGE ring (all transfers serialize on one DMA channel, so
    emission order w1a, x0, w1b, x1.. is the schedule).
  - the timing loop uses For_i(staggered_reset=True): the default
    all-engine barrier per iteration serializes the phases and costs
    ~6us/iter.

The K_* env vars below are experiment knobs; the defaults are the tuned
configuration and are what the graded kernel() path uses.

Sharding: data-parallel over batch. 512 rows / 8 cores = 64 rows per core,
processed in NH=4 chunks of 16 rows; weights replicated; no collectives.
"""

import contextlib
import sys

if "/opt/trn_rl_repo" not in sys.path:
    sys.path.insert(0, "/opt/trn_rl_repo")

import numpy as np

import concourse.bass as bass
import concourse.mybir as mybir
import concourse.tile as tile
from concourse import bacc
from concourse.bass_utils import run_bass_kernel_spmd

# Problem shape (hardcoded per contract).
B, C, F, H = 512, 16, 1024, 256
N_CORES = 8
BL = B // N_CORES          # 64 local batch rows per core
P = C * (C - 1) // 2       # 120 pairs
NH = 4                     # batch chunks per core
BH = BL // NH              # 16 rows per chunk
TOK = BL * C               # 1024 tokens per core
HTOK = BH * C              # 256 tokens per chunk; token = chunk*256 + c*16 + b
F32 = mybir.dt.float32
BF16 = mybir.dt.bfloat16
BF16_NP = mybir.dt.np(mybir.dt.bfloat16)

KT1 = F // 128             # 8 k-tiles for layer-1 contraction
PPG = 30                   # pairs per stage-C su-block
GW = PPG * BH              # su-block width: 480 columns
NG = P // PPG              # 4 su-blocks per half
NSP = NG // 2              # 2 psC tiles per half (2 su-blocks each)

import os


def _cfg():
    red = os.environ.get("K_REDUCERS", "pool2,pe,pe,pe").replace(
        "+", ","
    ).split(",")
    return {
        "reducer": {h: red[h] for h in range(4)},
        "n_warm": int(os.environ.get("K_WARM", "7")),
        "relu_on_act": os.environ.get("K_RELU_ACT", "0") == "1",
        "outdma": os.environ.get("K_OUTDMA", "sync"),
        # subtractive profiling: full | noE | noC | AB | A | dma
        "cut": os.environ.get("K_CUT", "full"),
        # number of trailing halves whose m=1 h2-drains go to DVE
        "dvedrain": int(os.environ.get("K_DVEDRAIN", "0")),
        # split the out-DMA into one transfer per E m-tile
        "splitout": os.environ.get("K_SPLITOUT", "0") == "1",
        # explicit staggered-reset stage boundaries at phase edges
        "sbound": os.environ.get("K_SBOUND", "0") == "1",
    }

# pair enumeration: for i in 0..C-2, j in i+1..C-1, p consecutive
PAIR_BASE = [0] * C
for _i in range(1, C):
    PAIR_BASE[_i] = PAIR_BASE[_i - 1] + (C - 1 - (_i - 1))

AF = mybir.ActivationFunctionType
ALU = mybir.AluOpType


def build_module(loop_iters: int = 1):
    cfg = _cfg()
    REDUCER = cfg["reducer"]
    N_WARM = cfg["n_warm"]
    RELU_ON_ACT = cfg["relu_on_act"]
    nc = bacc.Bacc("TRN2", target_bir_lowering=False, debug=True)

    xt_d = nc.dram_tensor("xt", [F, TOK], BF16, kind="ExternalInput")
    w1_d = nc.dram_tensor("w1", [2 * F, H], BF16, kind="ExternalInput")
    w2_d = nc.dram_tensor("w2", [H, H], BF16, kind="ExternalInput")
    w3_d = nc.dram_tensor("w3", [H, H], F32, kind="ExternalInput")
    bp_d = nc.dram_tensor("bias_pack", [128, 6], F32, kind="ExternalInput")
    id_d = nc.dram_tensor("ident", [128, 128], BF16, kind="ExternalInput")
    out_d = nc.dram_tensor("outT", [H, BL], F32, kind="ExternalOutput")

    with tile.TileContext(nc) as tc:
        with (
            tc.tile_pool(name="xpool", bufs=1) as xpool,
            tc.tile_pool(name="wpool", bufs=1) as wpool,
            tc.tile_pool(name="ypool", bufs=1) as ypool,
            tc.tile_pool(name="hpool", bufs=1) as hpool,
            tc.tile_pool(name="spool", bufs=1) as spool,
            tc.tile_pool(name="psA", bufs=1, space="PSUM") as psA_pool,
            tc.tile_pool(name="psC", bufs=2, space="PSUM") as psC_pool,
            tc.tile_pool(name="psR", bufs=2, space="PSUM") as psR_pool,
        ):
            loop_cm = (
                tc.For_i(
                    0, loop_iters, 1,
                    staggered_reset=os.environ.get("K_STAGGER", "1") == "1",
                )
                if loop_iters > 1
                else contextlib.nullcontext()
            )
            with loop_cm:
                # x chunk tiles
                xch = [
                    xpool.tile([128, KT1, HTOK], BF16, tag=f"x{h}", name=f"x{h}")
                    for h in range(NH)
                ]
                w1a = wpool.tile([128, KT1, H], BF16, tag="w1a", name="w1a")
                w1b = wpool.tile([128, KT1, H], BF16, tag="w1b", name="w1b")
                w2t = wpool.tile([128, 2, H], BF16, tag="w2t", name="w2t")
                w3t = wpool.tile([128, 2, H], F32, tag="w3t", name="w3t")
                bp = wpool.tile([128, 6], F32, tag="bp", name="bp")
                idt = wpool.tile([128, 128], BF16, tag="idt", name="idt")
                wsrc = wpool.tile([128, 16], BF16, tag="wsrc", name="wsrc")
                # per-half y tiles, free layout [m(4), c(C), b(BH)]
                yh = [
                    ypool.tile([128, 4, HTOK], BF16, tag=f"y{h}", name=f"y{h}")
                    for h in range(NH)
                ]
                # per-half h1 tiles, free layout [t(2), p(P), b(BH)]
                h1h = [
                    hpool.tile([128, 2, P * BH], BF16, tag=f"h1_{h}", name=f"h1_{h}")
                    for h in range(NH)
                ]
                # h2 tiles split per sp (su-block pair) for finer deps
                h2sb = [
                    [
                        [
                            spool.tile(
                                [128, 2 * GW], BF16,
                                tag=f"h2_{m}_{par}_{sp}",
                                name=f"h2_{m}_{par}_{sp}",
                            )
                            for sp in range(NSP)
                        ]
                        for par in range(2)
                    ]
                    for m in range(2)
                ]
                # scratch for the Pool 2:1 tree level, per (m, parity)
                tr2 = [
                    [
                        spool.tile(
                            [128, 2 * GW], BF16,
                            tag=f"tr2_{m}_{par}", name=f"tr2_{m}_{par}",
                        )
                        for par in range(2)
                    ]
                    for m in range(2)
                ]
                m2 = [
                    spool.tile([128, BL], F32, tag=f"m2_{m}", name=f"m2_{m}")
                    for m in range(2)
                ]
                osb = spool.tile([128, 2, BL], F32, tag="osb", name="osb")

                def bias(nm, t):
                    idx = {"b1": 0, "b2": 2, "b3": 4}[nm] + t
                    return bp[:, idx : idx + 1]

                # DRAM views
                w1v = w1_d.rearrange("(k p) h -> p k h", p=128)
                xtv = xt_d.rearrange("(k p) t -> p k t", p=128)

                def hs(half):
                    return slice(half * HTOK, (half + 1) * HTOK)

                # ---- DMA: weights/smalls on sync HWDGE; x on SWDGE ring.
                # All transfers serialize on one DMA channel, so the order
                # here is the transfer schedule: w1a first, x0 next (stage-A
                # critical path), then alternate weights with x chunks. ----
                nc.sync.dma_start(out=w1a[:], in_=w1v[:, 0:KT1, :])
                nc.sync.dma_start(out=bp[:], in_=bp_d[:])
                nc.sync.dma_start(out=w1b[:], in_=w1v[:, KT1 : 2 * KT1, :])
                nc.sync.dma_start(
                    out=w2t[:], in_=w2_d.rearrange("(k p) h -> p k h", p=128)
                )
                nc.sync.dma_start(out=idt[:], in_=id_d[:])
                nc.sync.dma_start(
                    out=w3t[:], in_=w3_d.rearrange("(k p) h -> p k h", p=128)
                )
                for ch in range(NH):
                    nc.gpsimd.dma_start(out=xch[ch][:], in_=xtv[:, :, hs(ch)])

                # PE warm-up from t~0: memset a small bf16 source, then
                # dummy matmuls to ramp the PE clock while DMAs stream.
                # Kept short: in the timing loop PE stays hot across
                # iterations and these are pure overhead.
                nc.vector.memset(wsrc[:], 0)
                warm = psC_pool.tile([128, 1024], F32, tag="psC", name="warm")
                for _ in range(N_WARM):
                    nc.tensor.matmul(
                        warm[:1, :512],
                        wsrc[:, 0:1],
                        wsrc[:, 0:1].broadcast_to([128, 512]),
                        start=True,
                        stop=True,
                    )

                def stage_A_piece(half, g, mt, psg):
                    """One layer-1 accumulation chain (8 matmuls)."""
                    w1g = w1a if g == 0 else w1b
                    for k in range(KT1):
                        nc.tensor.matmul(
                            psg[:, mt, :],
                            w1g[:, k, mt * 128 : (mt + 1) * 128],
                            xch[half][:, k, :],
                            start=(k == 0),
                            stop=(k == KT1 - 1),
                        )

                def stage_A_pieces(half):
                    """Layer-1 work for chunk `half` as 4 schedulable pieces
                    (2 chains per g; ACT drain after each g's second chain)."""
                    pieces = []
                    for g in range(2):
                        psg = psA_pool.tile(
                            [128, 2, HTOK], F32, tag=f"psA{g}",
                            name=f"psA{g}_{half}",
                        )

                        def mk(g=g, psg=psg, mt=0):
                            stage_A_piece(half, g, mt, psg)

                        def mk_last(g=g, psg=psg):
                            stage_A_piece(half, g, 1, psg)
                            nc.scalar.copy(
                                yh[half][:, 2 * g : 2 * g + 2, :], psg[:]
                            )

                        pieces.append(mk)
                        pieces.append(mk_last)
                    return pieces

                def stage_A(half):
                    for piece in stage_A_pieces(half):
                        piece()

                def stage_B(half):
                    """Pair-add on DVE (bf16 2x), then fused relu(h1+b1) via
                    4x-mode tensor_scalar per t."""
                    y5 = yh[half].rearrange("p m (c b) -> p m c b", b=BH)
                    h5 = h1h[half].rearrange("p t (pp b) -> p t pp b", b=BH)
                    for i in range(C - 1):
                        nj = C - 1 - i
                        p0 = PAIR_BASE[i]
                        in0 = y5[:, 0:2, i : i + 1, :].broadcast_to(
                            [128, 2, nj, BH]
                        )
                        in1 = y5[:, 2:4, i + 1 :, :]
                        outap = h5[:, :, p0 : p0 + nj, :]
                        nc.vector.tensor_add(outap, in0, in1)
                    for t in range(2):
                        sl = h1h[half][:, t, :]
                        if RELU_ON_ACT:
                            nc.scalar.activation(
                                sl, sl, AF.Relu, bias=bias("b1", t)
                            )
                        else:
                            nc.vector.tensor_scalar(
                                sl, sl, bias("b1", t), 0.0, ALU.add, ALU.max
                            )

                def stage_C(half, fillers=()):
                    """Layer-2 matmuls + ACT relu(+b2) drain into h2sb.
                    After each psC tile's matmuls, one pending filler (extra
                    PE work) is emitted to cover the ACT drain latency."""
                    fillers = list(fillers)
                    par = half % 2
                    for sp in range(NSP):
                        for m in range(2):
                            ps = psC_pool.tile(
                                [128, 1024], F32, tag="psC",
                                name=f"psC_{half}_{m}_{sp}",
                            )
                            for sub in range(2):
                                s = sp * 2 + sub
                                for k in range(2):
                                    nc.tensor.matmul(
                                        ps[:, sub * 512 : sub * 512 + GW],
                                        w2t[:, k, m * 128 : (m + 1) * 128],
                                        h1h[half][
                                            :, k, s * GW : (s + 1) * GW
                                        ],
                                        start=(k == 0),
                                        stop=(k == 1),
                                    )
                            h2t = h2sb[m][par][sp][:, :].rearrange(
                                "p (u g) -> p u g", g=GW
                            )
                            psv = ps.rearrange("p (u g) -> p u g", g=512)[:, :, :GW]
                            if m == 1 and half >= NH - cfg["dvedrain"]:
                                nc.vector.tensor_scalar(
                                    h2t, psv, bias("b2", m), 0.0,
                                    ALU.add, ALU.max,
                                )
                            else:
                                nc.scalar.activation(
                                    h2t, psv, AF.Relu, bias=bias("b2", m)
                                )
                            if fillers:
                                fillers.pop(0)()
                    for f in fillers:
                        f()

                def reduce_unit(half, m):
                    """Pair-sum of h2 for one (half, m) -> m2 slice."""
                    par = half % 2
                    kind = REDUCER[half]
                    if True:
                        dst = m2[m][:, half * BH : (half + 1) * BH]
                        if kind == "pool2":
                            t = tr2[m][par]
                            nc.gpsimd.tensor_add(
                                t[:], h2sb[m][par][0][:], h2sb[m][par][1][:]
                            )
                            v = (
                                t.rearrange("q (pp b) -> q pp b", b=BH)
                                .transpose([0, 2, 1])
                            )
                            nc.vector.tensor_reduce(
                                dst, v, mybir.AxisListType.X, ALU.add
                            )
                        else:  # pe: identity compaction + short DVE reduce
                            psr = psR_pool.tile(
                                [128, 512], F32, tag="psR", name=f"psR_{half}_{m}"
                            )
                            for su in range(NG):
                                nc.tensor.matmul(
                                    psr[:, :GW],
                                    idt[:],
                                    h2sb[m][par][su // 2][
                                        :, (su % 2) * GW : (su % 2 + 1) * GW
                                    ],
                                    start=(su == 0),
                                    stop=(su == NG - 1),
                                )
                            v = (
                                psr[:, :GW]
                                .rearrange("q (pp b) -> q pp b", b=BH)
                                .transpose([0, 2, 1])
                            )
                            nc.vector.tensor_reduce(
                                dst, v, mybir.AxisListType.X, ALU.add
                            )

                def reduce_half(half):
                    for m in range(2):
                        reduce_unit(half, m)

                # ---- pipeline: PE order A0 A1 A2 C0[A3] C1 C2 C3[R2] R3 E;
                # A3 fills C0's drain gaps, comp(2) fills C3's ----
                cut = cfg["cut"]

                def pe_fillers(h):
                    # interleave half h's PE-identity reduce units into the
                    # following C phase's drain gaps
                    if h >= 0 and REDUCER[h] == "pe":
                        return [
                            lambda: reduce_unit(h, 0),
                            lambda: reduce_unit(h, 1),
                        ]
                    return []

                sbound = (
                    cfg["sbound"] and loop_iters > 1
                    and os.environ.get("K_STAGGER", "0") == "1"
                )

                def sb():
                    if sbound:
                        tc.stage_boundary()

                if cut in ("full", "noE", "noC", "AB", "A"):
                    stage_A(0)
                    stage_A(1)
                    sb()
                    if cut != "A":
                        stage_B(0)
                    stage_A(2)
                    if cut != "A":
                        stage_B(1)
                    if cut in ("full", "noE"):
                        stage_C(0, fillers=stage_A_pieces(3))
                    else:
                        stage_A(3)
                    sb()
                    if cut != "A":
                        stage_B(2)
                        stage_B(3)
                    if cut in ("full", "noE"):
                        # pe-reduce fillers for half h may only interleave
                        # into stage_C(h+1): same-parity C(h+2) overwrites
                        # the h2 tiles that comp(h) reads
                        stage_C(1, fillers=pe_fillers(0))
                        if REDUCER[0] != "pe":
                            reduce_half(0)
                        sb()
                        stage_C(2, fillers=pe_fillers(1))
                        if REDUCER[1] != "pe":
                            reduce_half(1)
                        stage_C(3, fillers=pe_fillers(2))
                        if REDUCER[2] != "pe":
                            reduce_half(2)
                        reduce_half(3)
                    else:
                        sb()
                else:
                    sb()
                    sb()
                    sb()

                # ---- stage E: outT = (m2 @ W3scaled) + b3 (bias on DVE) ----
                if cut != "full":
                    nc.vector.memset(osb[:], 0)
                outeng = nc.sync if cfg["outdma"] == "sync" else nc.scalar
                odv = out_d.rearrange("(m p) b -> p m b", p=128)
                for mo in range(2) if cut == "full" else []:
                    ps = psR_pool.tile([128, 512], F32, tag="psR", name=f"psE_{mo}")
                    po = ps[:, :BL]
                    for k in range(2):
                        nc.tensor.matmul(
                            po,
                            w3t[:, k, mo * 128 : (mo + 1) * 128],
                            m2[k][:],
                            start=(k == 0),
                            stop=(k == 1),
                        )
                    nc.vector.tensor_scalar_add(osb[:, mo, :], po, bias("b3", mo))
                    if cfg["splitout"]:
                        outeng.dma_start(
                            out=odv[:, mo, :], in_=osb[:, mo, :]
                        )
                if cut != "full" or not cfg["splitout"]:
                    outeng.dma_start(out=odv[:], in_=osb[:])

    nc.compile()
    return nc


_NC_CACHE = None


def _get_module():
    global _NC_CACHE
    if _NC_CACHE is None:
        _NC_CACHE = build_module()
    return _NC_CACHE


def make_in_maps(x, W1, b1, W2, b2, W3, b3):
    W1 = np.ascontiguousarray(np.asarray(W1, dtype=np.float32).astype(BF16_NP))
    W2 = np.ascontiguousarray(np.asarray(W2, dtype=np.float32).astype(BF16_NP))
    w3p = np.ascontiguousarray(W3, dtype=np.float32) / np.float32(P)
    b1 = np.asarray(b1, dtype=np.float32)
    b2 = np.asarray(b2, dtype=np.float32)
    b3 = np.asarray(b3, dtype=np.float32)
    bias_pack = np.stack(
        [b1[:128], b1[128:], b2[:128], b2[128:], b3[:128], b3[128:]], axis=1
    )
    bias_pack = np.ascontiguousarray(bias_pack, dtype=np.float32)
    ident = np.eye(128, dtype=np.float32).astype(BF16_NP)
    in_maps = []
    for i in range(N_CORES):
        xs = x[i * BL : (i + 1) * BL]  # [BL, C, F]
        halves = [
            xs[h * BH : (h + 1) * BH].transpose(1, 0, 2).reshape(HTOK, F)
            for h in range(NH)
        ]
        xT = np.ascontiguousarray(
            np.concatenate(halves, axis=0).T.astype(BF16_NP)
        )
        in_maps.append(
            {
                "xt": xT,
                "w1": W1,
                "w2": W2,
                "w3": np.ascontiguousarray(w3p, dtype=np.float32),
                "bias_pack": bias_pack,
                "ident": ident,
            }
        )
    return in_maps


def kernel(x, W1, b1, W2, b2, W3, b3):
    nc = _get_module()
    in_maps = make_in_maps(
        np.asarray(x, dtype=np.float32),
        np.asarray(W1),
        np.asarray(b1),
        np.asarray(W2),
        np.asarray(b2),
        np.asarray(W3),
        np.asarray(b3),
    )
    res = run_bass_kernel_spmd(nc, in_maps, list(range(N_CORES)))
    out = np.empty((B, H), dtype=np.float32)
    for i in range(N_CORES):
        out[i * BL : (i + 1) * BL] = res.results[i]["outT"].T
    return out
